# revision 51
# baseline (speedup 1.0000x reference)
"""Bass/Tile Trainium2 kernel for nn_Attention_7284264534326.

Single-head attention, B=8, S=2048, D=1024:
    q = (x1 @ wq) * D**-0.5 ; k = x2 @ wk ; v = x2 @ wv
    a = softmax(q @ k^T + mask * -1e9, axis=-1)
    out = relu(a @ v) @ wo

Sharding: data-parallel over batch; one batch element per NeuronCore (8 cores).
Each core runs the identical program on its own x1[b], x2[b], maskSeq[b].

Per-core dataflow (all matmul operands bf16, PSUM accumulation f32):
  - x1, x2 are PE-transposed into X1T/X2T ([d, s] layout, bf16).
  - qT[e,s] = wq^T-free matmul(lhsT=wq[d,e], rhs=X1T[d,s]) with 1/32 scale on
    PSUM evacuation; kT likewise; V[s,e] = matmul(lhsT=X2T[d,s], rhs=wv[d,e]).
  - scores^T[k,q] = matmul(lhsT=kT, rhs=qT); exp fused into ACT evacuation
    with the additive mask as a per-partition bias (exp(-1e9) == 0).
  - yU^T[e,q] = matmul(lhsT=V, rhs=exp^T); denom via matmul with ones-vector.
  - z^T = relu(yU^T) (normalization deferred: relu(y/d) == relu(y)/d for d>0).
  - out[q,f] = matmul(lhsT=z^T, rhs=wo) scaled by 1/denom on evacuation.
"""

import numpy as np
from contextlib import ExitStack

B, S, D = 8, 2048, 1024
P = 128
DC = D // P       # 8 chunks of the depth/contraction dim
EC = D // P       # 8 chunks of the embedding dim
SC = S // P       # 16 chunks of the sequence dim
Q_TILE = 256      # queries per attention tile
NQT = S // Q_TILE # 8
S_SLICE = 256     # seq rows per projection slice
NSL = S // S_SLICE
N_CORES = 8
QSCALE = float(D) ** -0.5  # 1/32

_cached_nc = None


def _build():
    import concourse.tile as tile
    from concourse import bacc, mybir
    from concourse.masks import make_identity

    f32 = mybir.dt.float32
    bf16 = mybir.dt.bfloat16
    i32 = mybir.dt.int32
    AF = mybir.ActivationFunctionType

    nc = bacc.Bacc("TRN2", target_bir_lowering=False, debug=False,
                   enable_asserts=False, num_devices=N_CORES)

    x1 = nc.dram_tensor("x1", [S, D], bf16, kind="ExternalInput").ap()
    x2 = nc.dram_tensor("x2", [S, D], bf16, kind="ExternalInput").ap()
    msk = nc.dram_tensor("msk", [1, S], i32, kind="ExternalInput").ap()
    wq = nc.dram_tensor("wq", [D, D], bf16, kind="ExternalInput").ap()
    wk = nc.dram_tensor("wk", [D, D], bf16, kind="ExternalInput").ap()
    wv = nc.dram_tensor("wv", [D, D], bf16, kind="ExternalInput").ap()
    wo = nc.dram_tensor("wo", [D, D], bf16, kind="ExternalInput").ap()
    out = nc.dram_tensor("out", [S, D], f32, kind="ExternalOutput").ap()

    with tile.TileContext(nc) as tc, ExitStack() as ctx:
        persist = ctx.enter_context(tc.tile_pool(name="persist", bufs=1))

        qT = persist.tile([P, EC, S], bf16, name="qT")        # [e, s] by e-chunk
        kT = persist.tile([P, EC, S], bf16, name="kT")
        V = persist.tile([P, SC, D], bf16, name="V")          # [s, e] by s-chunk
        ident = persist.tile([P, P], f32, name="ident")
        ident_bf = persist.tile([P, P], bf16, name="ident_bf")
        ones_bf = persist.tile([P, 1], bf16, name="ones_bf")
        maskbias = persist.tile([P, SC], f32, name="maskbias")

        make_identity(nc, ident_bf)
        nc.vector.memset(ones_bf, 1.0)

        def emit_mask_prep(psum_pool):
            # mask -> per-partition additive bias, [s] laid out as [P, SC]
            make_identity(nc, ident)
            with tc.tile_pool(name="mprep", bufs=1) as mprep:
                mint = mprep.tile([SC, P], i32, name="mint")
                nc.sync.dma_start(mint, msk.rearrange("o (c p) -> (o c) p", p=P))
                mf = mprep.tile([SC, P], f32, name="mf")
                nc.vector.tensor_copy(out=mf, in_=mint)       # int32 -> f32
                mneg = mprep.tile([SC, P], f32, name="mneg")
                nc.scalar.mul(mneg, mf, -1.0e9)
                mp = psum_pool.tile([P, SC], f32, name="mp", tag="pt")
                nc.tensor.transpose(mp, mneg, ident[:SC, :SC])  # [SC,P]->[P,SC]
                nc.vector.tensor_copy(out=maskbias, in_=mp)

        # ================= phase 1: projections =================
        with ExitStack() as pctx:
            wpool = pctx.enter_context(tc.tile_pool(name="wpool", bufs=1))
            xstage = pctx.enter_context(tc.tile_pool(name="xstage", bufs=2))
            xtpool = pctx.enter_context(tc.tile_pool(name="xtpool", bufs=5))
            tpsum = pctx.enter_context(tc.tile_pool(name="tpsum", bufs=4, space="PSUM"))
            ppsum = pctx.enter_context(tc.tile_pool(name="ppsum", bufs=4, space="PSUM"))

            # weights arrive as bf16 from the host: direct DMA into place,
            # wq on the scalar ring, wk/wv on gpsimd — parallel to x on sync.
            wq_bf = wpool.tile([P, DC, D], bf16, name="wq_bf")
            wk_bf = wpool.tile([P, DC, D], bf16, name="wk_bf")
            wv_bf = wpool.tile([P, DC, D], bf16, name="wv_bf")
            # each weight split across two rings so every projection's weight
            # lands early; sync stays x-first (slice-0 x DMAs emitted below
            # race ahead only on the scalar/gpsimd/vector rings).
            for dc in range(DC):
                nc.scalar.dma_start(wq_bf[:, dc, :], wq[dc * P:(dc + 1) * P, :])
                nc.gpsimd.dma_start(wk_bf[:, dc, :], wk[dc * P:(dc + 1) * P, :])
            for dc in range(DC):
                eng = nc.scalar if dc % 2 == 0 else nc.gpsimd
                eng.dma_start(wv_bf[:, dc, :], wv[dc * P:(dc + 1) * P, :])

            xts = {}

            def emit_transposes(sl):
                s0 = sl * S_SLICE
                x1t = xtpool.tile([P, DC, S_SLICE], bf16, name="x1t", tag="x1t")
                x2t = xtpool.tile([P, DC, S_SLICE], bf16, name="x2t", tag="x2t")
                xts[sl] = (x1t, x2t)
                for j in range(S_SLICE // P):
                    for x_ap, xt, nm in ((x1, x1t, "x1s"), (x2, x2t, "x2s")):
                        xs = xstage.tile([P, D], bf16, name=nm, tag="xs", bufs=8)
                        nc.sync.dma_start(xs, x_ap[s0 + j * P: s0 + (j + 1) * P, :])
                        for dq in range(DC // 4):
                            # 4 transposes share one psum tile -> 1 DVE evac.
                            # Regular matmul vs identity (out = x_blk.T @ I)
                            # streams through the normal MM pipeline — faster
                            # than transpose-mode and it keeps HAM warm.
                            pt = tpsum.tile([P, 4, P], f32, name="pt", tag="pt")
                            for di in range(4):
                                dc = dq * 4 + di
                                nc.tensor.matmul(
                                    pt[:, di, :],
                                    lhsT=xs[:, dc * P:(dc + 1) * P],
                                    rhs=ident_bf, start=True, stop=True)
                            nc.vector.tensor_copy(
                                out=xt[:, dq * 4:(dq + 1) * 4, j * P:(j + 1) * P],
                                in_=pt)

            def emit_projections(sl):
                s0 = sl * S_SLICE
                x1t, x2t = xts.pop(sl)
                for w_bf, dstT, src, scale in ((wq_bf, qT, x1t, QSCALE),
                                               (wk_bf, kT, x2t, None)):
                    for ec in range(EC):
                        pq = ppsum.tile([P, S_SLICE], f32, name="pq", tag="pp")
                        for dc in range(DC):
                            nc.tensor.matmul(
                                pq, lhsT=w_bf[:, dc, ec * P:(ec + 1) * P],
                                rhs=src[:, dc, :],
                                start=(dc == 0), stop=(dc == DC - 1))
                        if scale is not None:
                            nc.scalar.activation(
                                out=dstT[:, ec, s0:s0 + S_SLICE], in_=pq,
                                func=AF.Copy, scale=scale)
                        else:
                            nc.scalar.activation(
                                out=dstT[:, ec, s0:s0 + S_SLICE], in_=pq,
                                func=AF.Copy)
                for j in range(S_SLICE // P):
                    for eh in range(2):
                        pv = ppsum.tile([P, 512], f32, name="pv", tag="pp")
                        for dc in range(DC):
                            nc.tensor.matmul(
                                pv, lhsT=x2t[:, dc, j * P:(j + 1) * P],
                                rhs=wv_bf[:, dc, eh * 512:(eh + 1) * 512],
                                start=(dc == 0), stop=(dc == DC - 1))
                        nc.scalar.activation(
                            out=V[:, sl * (S_SLICE // P) + j, eh * 512:(eh + 1) * 512],
                            in_=pv, func=AF.Copy)

            # transposes (no weight dependency) run LAG slices ahead so the
            # PE has work while the weights stream in.
            LAG = 3
            for sl in range(NSL + LAG):
                if sl < NSL:
                    emit_transposes(sl)
                if sl == 0:
                    emit_mask_prep(tpsum)
                if sl >= LAG:
                    emit_projections(sl - LAG)

        # ================= phase 2: attention =================
        wopool = ctx.enter_context(tc.tile_pool(name="wopool", bufs=1))
        wo_bf = wopool.tile([P, DC, D], bf16, name="wo_bf")
        epool = ctx.enter_context(tc.tile_pool(name="epool", bufs=3))
        zpool = ctx.enter_context(tc.tile_pool(name="zpool", bufs=2))
        opool = ctx.enter_context(tc.tile_pool(name="opool", bufs=3))
        rpool = ctx.enter_context(tc.tile_pool(name="rpool", bufs=2))
        spsum = ctx.enter_context(tc.tile_pool(name="spsum", bufs=2, space="PSUM"))
        ypsum = ctx.enter_context(tc.tile_pool(name="ypsum", bufs=3, space="PSUM"))
        dpsum = ctx.enter_context(tc.tile_pool(name="dpsum", bufs=1, space="PSUM"))
        opsum = ctx.enter_context(tc.tile_pool(name="opsum", bufs=2, space="PSUM"))

        for dc in range(DC):
            nc.gpsimd.dma_start(wo_bf[:, dc, :], wo[dc * P:(dc + 1) * P, :])

        for qt in range(NQT):
            q0 = qt * Q_TILE
            expt = epool.tile([P, SC, Q_TILE], bf16, name="expt", tag="expt")
            for kc in range(SC):
                ps = spsum.tile([P, Q_TILE], f32, name="ps", tag="ps")
                for ec in range(EC):
                    nc.tensor.matmul(
                        ps, lhsT=kT[:, ec, kc * P:(kc + 1) * P],
                        rhs=qT[:, ec, q0:q0 + Q_TILE],
                        start=(ec == 0), stop=(ec == EC - 1))
                nc.scalar.activation(
                    out=expt[:, kc, :], in_=ps, func=AF.Exp,
                    bias=maskbias[:, kc:kc + 1], scale=1.0)

            # NOTE: matmul start=True clears has_written bits for the WHOLE
            # PSUM bank, so accumulation chains must not interleave within a
            # bank: run one chain per psum tile to completion before starting
            # the next chain that shares its bank.
            zt = zpool.tile([P, EC, Q_TILE], bf16, name="zt", tag="zt")
            for ec in range(EC):
                py = ypsum.tile([P, Q_TILE], f32, name="py", tag="py")
                for kc in range(SC):
                    nc.tensor.matmul(
                        py, lhsT=V[:, kc, ec * P:(ec + 1) * P],
                        rhs=expt[:, kc, :],
                        start=(kc == 0), stop=(kc == SC - 1))
                nc.scalar.activation(out=zt[:, ec, :], in_=py, func=AF.Relu)

            pd = dpsum.tile([P, Q_TILE // P], f32, name="pd", tag="pd")
            for qs in range(Q_TILE // P):
                for kc in range(SC):
                    nc.tensor.matmul(
                        pd[:, qs:qs + 1],
                        lhsT=expt[:, kc, qs * P:(qs + 1) * P], rhs=ones_bf,
                        start=(kc == 0), stop=(kc == SC - 1))
            recip = rpool.tile([P, Q_TILE // P], f32, name="recip", tag="recip")
            nc.vector.reciprocal(recip, pd)

            for qs in range(Q_TILE // P):
                osb = opool.tile([P, D], f32, name="osb", tag="osb")
                for fh in range(2):
                    po = opsum.tile([P, 512], f32, name="po", tag="po")
                    for ec in range(EC):
                        nc.tensor.matmul(
                            po, lhsT=zt[:, ec, qs * P:(qs + 1) * P],
                            rhs=wo_bf[:, ec, fh * 512:(fh + 1) * 512],
                            start=(ec == 0), stop=(ec == EC - 1))
                    nc.scalar.activation(
                        out=osb[:, fh * 512:(fh + 1) * 512], in_=po,
                        func=AF.Copy, scale=recip[:, qs:qs + 1])
                    # per-half store: the DMA overlaps the other half's evac
                    nc.sync.dma_start(
                        out[q0 + qs * P: q0 + (qs + 1) * P,
                            fh * 512:(fh + 1) * 512],
                        osb[:, fh * 512:(fh + 1) * 512])

    nc.compile()
    return nc


def kernel(x1, x2, maskSeq, wq, wk, wv, wo, **_unused):
    from concourse.bass_utils import run_bass_kernel_spmd

    global _cached_nc
    if _cached_nc is None:
        _cached_nc = _build()
    nc = _cached_nc

    import ml_dtypes
    bf = ml_dtypes.bfloat16
    x1 = np.ascontiguousarray(np.asarray(x1, dtype=np.float32).astype(bf))
    x2 = np.ascontiguousarray(np.asarray(x2, dtype=np.float32).astype(bf))
    maskSeq = np.ascontiguousarray(np.asarray(maskSeq, dtype=np.int32))
    wq = np.ascontiguousarray(np.asarray(wq, dtype=np.float32).astype(bf))
    wk = np.ascontiguousarray(np.asarray(wk, dtype=np.float32).astype(bf))
    wv = np.ascontiguousarray(np.asarray(wv, dtype=np.float32).astype(bf))
    wo = np.ascontiguousarray(np.asarray(wo, dtype=np.float32).astype(bf))

    in_maps = [
        {"x1": x1[c], "x2": x2[c], "msk": maskSeq[c],
         "wq": wq, "wk": wk, "wv": wv, "wo": wo}
        for c in range(N_CORES)
    ]
    res = run_bass_kernel_spmd(nc, in_maps, core_ids=list(range(N_CORES)))
    return np.stack([res.results[c]["out"] for c in range(N_CORES)], axis=0)


# revision 52
# speedup vs baseline: 1.0014x; 1.0014x over previous
"""Bass/Tile Trainium2 kernel for nn_Attention_7284264534326.

Single-head attention, B=8, S=2048, D=1024:
    q = (x1 @ wq) * D**-0.5 ; k = x2 @ wk ; v = x2 @ wv
    a = softmax(q @ k^T + mask * -1e9, axis=-1)
    out = relu(a @ v) @ wo

Sharding: data-parallel over batch; one batch element per NeuronCore (8 cores).
Each core runs the identical program on its own x1[b], x2[b], maskSeq[b].

Per-core dataflow (all matmul operands bf16, PSUM accumulation f32):
  - x1, x2 are PE-transposed into X1T/X2T ([d, s] layout, bf16).
  - qT[e,s] = wq^T-free matmul(lhsT=wq[d,e], rhs=X1T[d,s]) with 1/32 scale on
    PSUM evacuation; kT likewise; V[s,e] = matmul(lhsT=X2T[d,s], rhs=wv[d,e]).
  - scores^T[k,q] = matmul(lhsT=kT, rhs=qT); exp fused into ACT evacuation
    with the additive mask as a per-partition bias (exp(-1e9) == 0).
  - yU^T[e,q] = matmul(lhsT=V, rhs=exp^T); denom via matmul with ones-vector.
  - z^T = relu(yU^T) (normalization deferred: relu(y/d) == relu(y)/d for d>0).
  - out[q,f] = matmul(lhsT=z^T, rhs=wo) scaled by 1/denom on evacuation.
"""

import numpy as np
from contextlib import ExitStack

B, S, D = 8, 2048, 1024
P = 128
DC = D // P       # 8 chunks of the depth/contraction dim
EC = D // P       # 8 chunks of the embedding dim
SC = S // P       # 16 chunks of the sequence dim
Q_TILE = 256      # queries per attention tile
NQT = S // Q_TILE # 8
S_SLICE = 256     # seq rows per projection slice
NSL = S // S_SLICE
N_CORES = 8
QSCALE = float(D) ** -0.5  # 1/32

_cached_nc = None


def _build():
    import concourse.tile as tile
    from concourse import bacc, mybir
    from concourse.masks import make_identity

    f32 = mybir.dt.float32
    bf16 = mybir.dt.bfloat16
    i32 = mybir.dt.int32
    AF = mybir.ActivationFunctionType

    nc = bacc.Bacc("TRN2", target_bir_lowering=False, debug=False,
                   enable_asserts=False, num_devices=N_CORES)

    x1 = nc.dram_tensor("x1", [S, D], bf16, kind="ExternalInput").ap()
    x2 = nc.dram_tensor("x2", [S, D], bf16, kind="ExternalInput").ap()
    msk = nc.dram_tensor("msk", [1, S], i32, kind="ExternalInput").ap()
    wq = nc.dram_tensor("wq", [D, D], bf16, kind="ExternalInput").ap()
    wk = nc.dram_tensor("wk", [D, D], bf16, kind="ExternalInput").ap()
    wv = nc.dram_tensor("wv", [D, D], bf16, kind="ExternalInput").ap()
    wo = nc.dram_tensor("wo", [D, D], bf16, kind="ExternalInput").ap()
    out = nc.dram_tensor("out", [S, D], f32, kind="ExternalOutput").ap()

    with tile.TileContext(nc) as tc, ExitStack() as ctx:
        persist = ctx.enter_context(tc.tile_pool(name="persist", bufs=1))

        qT = persist.tile([P, EC, S], bf16, name="qT")        # [e, s] by e-chunk
        kT = persist.tile([P, EC, S], bf16, name="kT")
        V = persist.tile([P, SC, D], bf16, name="V")          # [s, e] by s-chunk
        ident = persist.tile([P, P], f32, name="ident")
        ident_bf = persist.tile([P, P], bf16, name="ident_bf")
        ones_bf = persist.tile([P, 1], bf16, name="ones_bf")
        maskbias = persist.tile([P, SC], f32, name="maskbias")

        make_identity(nc, ident_bf)
        nc.vector.memset(ones_bf, 1.0)

        def emit_mask_prep(psum_pool):
            # mask -> per-partition additive bias, [s] laid out as [P, SC]
            make_identity(nc, ident)
            with tc.tile_pool(name="mprep", bufs=1) as mprep:
                mint = mprep.tile([SC, P], i32, name="mint")
                nc.sync.dma_start(mint, msk.rearrange("o (c p) -> (o c) p", p=P))
                mf = mprep.tile([SC, P], f32, name="mf")
                nc.vector.tensor_copy(out=mf, in_=mint)       # int32 -> f32
                mneg = mprep.tile([SC, P], f32, name="mneg")
                nc.scalar.mul(mneg, mf, -1.0e9)
                mp = psum_pool.tile([P, SC], f32, name="mp", tag="pt")
                nc.tensor.transpose(mp, mneg, ident[:SC, :SC])  # [SC,P]->[P,SC]
                nc.vector.tensor_copy(out=maskbias, in_=mp)

        # ================= phase 1: projections =================
        with ExitStack() as pctx:
            wpool = pctx.enter_context(tc.tile_pool(name="wpool", bufs=1))
            xstage = pctx.enter_context(tc.tile_pool(name="xstage", bufs=2))
            xtpool = pctx.enter_context(tc.tile_pool(name="xtpool", bufs=5))
            tpsum = pctx.enter_context(tc.tile_pool(name="tpsum", bufs=4, space="PSUM"))
            ppsum = pctx.enter_context(tc.tile_pool(name="ppsum", bufs=4, space="PSUM"))

            # weights arrive as bf16 from the host: direct DMA into place,
            # wq on the scalar ring, wk/wv on gpsimd — parallel to x on sync.
            wq_bf = wpool.tile([P, DC, D], bf16, name="wq_bf")
            wk_bf = wpool.tile([P, DC, D], bf16, name="wk_bf")
            wv_bf = wpool.tile([P, DC, D], bf16, name="wv_bf")
            # each weight split across two rings so every projection's weight
            # lands early; sync stays x-first (slice-0 x DMAs emitted below
            # race ahead only on the scalar/gpsimd/vector rings).
            for dc in range(DC):
                nc.scalar.dma_start(wq_bf[:, dc, :], wq[dc * P:(dc + 1) * P, :])
                nc.gpsimd.dma_start(wk_bf[:, dc, :], wk[dc * P:(dc + 1) * P, :])
            for dc in range(DC):
                eng = nc.scalar if dc % 2 == 0 else nc.gpsimd
                eng.dma_start(wv_bf[:, dc, :], wv[dc * P:(dc + 1) * P, :])

            xts = {}

            def emit_transposes(sl):
                s0 = sl * S_SLICE
                x1t = xtpool.tile([P, DC, S_SLICE], bf16, name="x1t", tag="x1t")
                x2t = xtpool.tile([P, DC, S_SLICE], bf16, name="x2t", tag="x2t")
                xts[sl] = (x1t, x2t)
                for j in range(S_SLICE // P):
                    for x_ap, xt, nm in ((x1, x1t, "x1s"), (x2, x2t, "x2s")):
                        xs = xstage.tile([P, D], bf16, name=nm, tag="xs", bufs=8)
                        nc.sync.dma_start(xs, x_ap[s0 + j * P: s0 + (j + 1) * P, :])
                        for dq in range(DC // 4):
                            # 4 transposes share one psum tile -> 1 DVE evac.
                            # Regular matmul vs identity (out = x_blk.T @ I)
                            # streams through the normal MM pipeline — faster
                            # than transpose-mode and it keeps HAM warm.
                            pt = tpsum.tile([P, 4, P], f32, name="pt", tag="pt")
                            for di in range(4):
                                dc = dq * 4 + di
                                nc.tensor.matmul(
                                    pt[:, di, :],
                                    lhsT=xs[:, dc * P:(dc + 1) * P],
                                    rhs=ident_bf, start=True, stop=True)
                            nc.vector.tensor_copy(
                                out=xt[:, dq * 4:(dq + 1) * 4, j * P:(j + 1) * P],
                                in_=pt)

            def emit_projections(sl):
                s0 = sl * S_SLICE
                x1t, x2t = xts.pop(sl)
                for w_bf, dstT, src, scale in ((wq_bf, qT, x1t, QSCALE),
                                               (wk_bf, kT, x2t, None)):
                    for ec in range(EC):
                        pq = ppsum.tile([P, S_SLICE], f32, name="pq", tag="pp")
                        for dc in range(DC):
                            nc.tensor.matmul(
                                pq, lhsT=w_bf[:, dc, ec * P:(ec + 1) * P],
                                rhs=src[:, dc, :],
                                start=(dc == 0), stop=(dc == DC - 1))
                        if scale is not None:
                            nc.scalar.activation(
                                out=dstT[:, ec, s0:s0 + S_SLICE], in_=pq,
                                func=AF.Copy, scale=scale)
                        else:
                            nc.scalar.activation(
                                out=dstT[:, ec, s0:s0 + S_SLICE], in_=pq,
                                func=AF.Copy)
                for j in range(S_SLICE // P):
                    for eh in range(2):
                        pv = ppsum.tile([P, 512], f32, name="pv", tag="pp")
                        for dc in range(DC):
                            nc.tensor.matmul(
                                pv, lhsT=x2t[:, dc, j * P:(j + 1) * P],
                                rhs=wv_bf[:, dc, eh * 512:(eh + 1) * 512],
                                start=(dc == 0), stop=(dc == DC - 1))
                        nc.scalar.activation(
                            out=V[:, sl * (S_SLICE // P) + j, eh * 512:(eh + 1) * 512],
                            in_=pv, func=AF.Copy)

            # transposes (no weight dependency) run LAG slices ahead so the
            # PE has work while the weights stream in.
            LAG = 2
            for sl in range(NSL + LAG):
                if sl < NSL:
                    emit_transposes(sl)
                if sl == 0:
                    emit_mask_prep(tpsum)
                if sl >= LAG:
                    emit_projections(sl - LAG)

        # ================= phase 2: attention =================
        wopool = ctx.enter_context(tc.tile_pool(name="wopool", bufs=1))
        wo_bf = wopool.tile([P, DC, D], bf16, name="wo_bf")
        epool = ctx.enter_context(tc.tile_pool(name="epool", bufs=3))
        zpool = ctx.enter_context(tc.tile_pool(name="zpool", bufs=2))
        opool = ctx.enter_context(tc.tile_pool(name="opool", bufs=3))
        rpool = ctx.enter_context(tc.tile_pool(name="rpool", bufs=2))
        spsum = ctx.enter_context(tc.tile_pool(name="spsum", bufs=2, space="PSUM"))
        ypsum = ctx.enter_context(tc.tile_pool(name="ypsum", bufs=3, space="PSUM"))
        dpsum = ctx.enter_context(tc.tile_pool(name="dpsum", bufs=1, space="PSUM"))
        opsum = ctx.enter_context(tc.tile_pool(name="opsum", bufs=2, space="PSUM"))

        for dc in range(DC):
            nc.gpsimd.dma_start(wo_bf[:, dc, :], wo[dc * P:(dc + 1) * P, :])

        for qt in range(NQT):
            q0 = qt * Q_TILE
            expt = epool.tile([P, SC, Q_TILE], bf16, name="expt", tag="expt")
            for kc in range(SC):
                ps = spsum.tile([P, Q_TILE], f32, name="ps", tag="ps")
                for ec in range(EC):
                    nc.tensor.matmul(
                        ps, lhsT=kT[:, ec, kc * P:(kc + 1) * P],
                        rhs=qT[:, ec, q0:q0 + Q_TILE],
                        start=(ec == 0), stop=(ec == EC - 1))
                nc.scalar.activation(
                    out=expt[:, kc, :], in_=ps, func=AF.Exp,
                    bias=maskbias[:, kc:kc + 1], scale=1.0)

            # NOTE: matmul start=True clears has_written bits for the WHOLE
            # PSUM bank, so accumulation chains must not interleave within a
            # bank: run one chain per psum tile to completion before starting
            # the next chain that shares its bank.
            zt = zpool.tile([P, EC, Q_TILE], bf16, name="zt", tag="zt")
            for ec in range(EC):
                py = ypsum.tile([P, Q_TILE], f32, name="py", tag="py")
                for kc in range(SC):
                    nc.tensor.matmul(
                        py, lhsT=V[:, kc, ec * P:(ec + 1) * P],
                        rhs=expt[:, kc, :],
                        start=(kc == 0), stop=(kc == SC - 1))
                nc.scalar.activation(out=zt[:, ec, :], in_=py, func=AF.Relu)

            pd = dpsum.tile([P, Q_TILE // P], f32, name="pd", tag="pd")
            for qs in range(Q_TILE // P):
                for kc in range(SC):
                    nc.tensor.matmul(
                        pd[:, qs:qs + 1],
                        lhsT=expt[:, kc, qs * P:(qs + 1) * P], rhs=ones_bf,
                        start=(kc == 0), stop=(kc == SC - 1))
            recip = rpool.tile([P, Q_TILE // P], f32, name="recip", tag="recip")
            nc.vector.reciprocal(recip, pd)

            for qs in range(Q_TILE // P):
                osb = opool.tile([P, D], f32, name="osb", tag="osb")
                for fh in range(2):
                    po = opsum.tile([P, 512], f32, name="po", tag="po")
                    for ec in range(EC):
                        nc.tensor.matmul(
                            po, lhsT=zt[:, ec, qs * P:(qs + 1) * P],
                            rhs=wo_bf[:, ec, fh * 512:(fh + 1) * 512],
                            start=(ec == 0), stop=(ec == EC - 1))
                    nc.scalar.activation(
                        out=osb[:, fh * 512:(fh + 1) * 512], in_=po,
                        func=AF.Copy, scale=recip[:, qs:qs + 1])
                    # per-half store: the DMA overlaps the other half's evac
                    nc.sync.dma_start(
                        out[q0 + qs * P: q0 + (qs + 1) * P,
                            fh * 512:(fh + 1) * 512],
                        osb[:, fh * 512:(fh + 1) * 512])

    nc.compile()
    return nc


def kernel(x1, x2, maskSeq, wq, wk, wv, wo, **_unused):
    from concourse.bass_utils import run_bass_kernel_spmd

    global _cached_nc
    if _cached_nc is None:
        _cached_nc = _build()
    nc = _cached_nc

    import ml_dtypes
    bf = ml_dtypes.bfloat16
    x1 = np.ascontiguousarray(np.asarray(x1, dtype=np.float32).astype(bf))
    x2 = np.ascontiguousarray(np.asarray(x2, dtype=np.float32).astype(bf))
    maskSeq = np.ascontiguousarray(np.asarray(maskSeq, dtype=np.int32))
    wq = np.ascontiguousarray(np.asarray(wq, dtype=np.float32).astype(bf))
    wk = np.ascontiguousarray(np.asarray(wk, dtype=np.float32).astype(bf))
    wv = np.ascontiguousarray(np.asarray(wv, dtype=np.float32).astype(bf))
    wo = np.ascontiguousarray(np.asarray(wo, dtype=np.float32).astype(bf))

    in_maps = [
        {"x1": x1[c], "x2": x2[c], "msk": maskSeq[c],
         "wq": wq, "wk": wk, "wv": wv, "wo": wo}
        for c in range(N_CORES)
    ]
    res = run_bass_kernel_spmd(nc, in_maps, core_ids=list(range(N_CORES)))
    return np.stack([res.results[c]["out"] for c in range(N_CORES)], axis=0)


# revision 53
# speedup vs baseline: 1.0028x; 1.0014x over previous
"""Bass/Tile Trainium2 kernel for nn_Attention_7284264534326.

Single-head attention, B=8, S=2048, D=1024:
    q = (x1 @ wq) * D**-0.5 ; k = x2 @ wk ; v = x2 @ wv
    a = softmax(q @ k^T + mask * -1e9, axis=-1)
    out = relu(a @ v) @ wo

Sharding: data-parallel over batch; one batch element per NeuronCore (8 cores).
Each core runs the identical program on its own x1[b], x2[b], maskSeq[b].

Per-core dataflow (all matmul operands bf16, PSUM accumulation f32):
  - x1, x2 are PE-transposed into X1T/X2T ([d, s] layout, bf16).
  - qT[e,s] = wq^T-free matmul(lhsT=wq[d,e], rhs=X1T[d,s]) with 1/32 scale on
    PSUM evacuation; kT likewise; V[s,e] = matmul(lhsT=X2T[d,s], rhs=wv[d,e]).
  - scores^T[k,q] = matmul(lhsT=kT, rhs=qT); exp fused into ACT evacuation
    with the additive mask as a per-partition bias (exp(-1e9) == 0).
  - yU^T[e,q] = matmul(lhsT=V, rhs=exp^T); denom via matmul with ones-vector.
  - z^T = relu(yU^T) (normalization deferred: relu(y/d) == relu(y)/d for d>0).
  - out[q,f] = matmul(lhsT=z^T, rhs=wo) scaled by 1/denom on evacuation.
"""

import numpy as np
from contextlib import ExitStack

B, S, D = 8, 2048, 1024
P = 128
DC = D // P       # 8 chunks of the depth/contraction dim
EC = D // P       # 8 chunks of the embedding dim
SC = S // P       # 16 chunks of the sequence dim
Q_TILE = 256      # queries per attention tile
NQT = S // Q_TILE # 8
S_SLICE = 256     # seq rows per projection slice
NSL = S // S_SLICE
N_CORES = 8
QSCALE = float(D) ** -0.5  # 1/32

_cached_nc = None


def _build():
    import concourse.tile as tile
    from concourse import bacc, mybir
    from concourse.masks import make_identity

    f32 = mybir.dt.float32
    bf16 = mybir.dt.bfloat16
    i32 = mybir.dt.int32
    AF = mybir.ActivationFunctionType

    nc = bacc.Bacc("TRN2", target_bir_lowering=False, debug=False,
                   enable_asserts=False, num_devices=N_CORES)

    x1 = nc.dram_tensor("x1", [S, D], bf16, kind="ExternalInput").ap()
    x2 = nc.dram_tensor("x2", [S, D], bf16, kind="ExternalInput").ap()
    msk = nc.dram_tensor("msk", [1, S], i32, kind="ExternalInput").ap()
    wq = nc.dram_tensor("wq", [D, D], bf16, kind="ExternalInput").ap()
    wk = nc.dram_tensor("wk", [D, D], bf16, kind="ExternalInput").ap()
    wv = nc.dram_tensor("wv", [D, D], bf16, kind="ExternalInput").ap()
    wo = nc.dram_tensor("wo", [D, D], bf16, kind="ExternalInput").ap()
    out = nc.dram_tensor("out", [S, D], f32, kind="ExternalOutput").ap()

    with tile.TileContext(nc) as tc, ExitStack() as ctx:
        persist = ctx.enter_context(tc.tile_pool(name="persist", bufs=1))

        qT = persist.tile([P, EC, S], bf16, name="qT")        # [e, s] by e-chunk
        kT = persist.tile([P, EC, S], bf16, name="kT")
        V = persist.tile([P, SC, D], bf16, name="V")          # [s, e] by s-chunk
        ident = persist.tile([P, P], f32, name="ident")
        ident_bf = persist.tile([P, P], bf16, name="ident_bf")
        ones_bf = persist.tile([P, 1], bf16, name="ones_bf")
        maskbias = persist.tile([P, SC], f32, name="maskbias")

        make_identity(nc, ident_bf)
        nc.vector.memset(ones_bf, 1.0)

        def emit_mask_prep(psum_pool):
            # mask -> per-partition additive bias, [s] laid out as [P, SC]
            make_identity(nc, ident)
            with tc.tile_pool(name="mprep", bufs=1) as mprep:
                mint = mprep.tile([SC, P], i32, name="mint")
                nc.sync.dma_start(mint, msk.rearrange("o (c p) -> (o c) p", p=P))
                mf = mprep.tile([SC, P], f32, name="mf")
                nc.vector.tensor_copy(out=mf, in_=mint)       # int32 -> f32
                mneg = mprep.tile([SC, P], f32, name="mneg")
                nc.scalar.mul(mneg, mf, -1.0e9)
                mp = psum_pool.tile([P, SC], f32, name="mp", tag="pt")
                nc.tensor.transpose(mp, mneg, ident[:SC, :SC])  # [SC,P]->[P,SC]
                nc.vector.tensor_copy(out=maskbias, in_=mp)

        # ================= phase 1: projections =================
        with ExitStack() as pctx:
            wpool = pctx.enter_context(tc.tile_pool(name="wpool", bufs=1))
            xstage = pctx.enter_context(tc.tile_pool(name="xstage", bufs=2))
            xtpool = pctx.enter_context(tc.tile_pool(name="xtpool", bufs=5))
            tpsum = pctx.enter_context(tc.tile_pool(name="tpsum", bufs=4, space="PSUM"))
            ppsum = pctx.enter_context(tc.tile_pool(name="ppsum", bufs=4, space="PSUM"))

            # weights arrive as bf16 from the host: direct DMA into place,
            # wq on the scalar ring, wk/wv on gpsimd — parallel to x on sync.
            wq_bf = wpool.tile([P, DC, D], bf16, name="wq_bf")
            wk_bf = wpool.tile([P, DC, D], bf16, name="wk_bf")
            wv_bf = wpool.tile([P, DC, D], bf16, name="wv_bf")
            # each weight split across two rings so every projection's weight
            # lands early; sync stays x-first (slice-0 x DMAs emitted below
            # race ahead only on the scalar/gpsimd/vector rings).
            nc.scalar.dma_start(wq_bf, wq.rearrange("(c p) e -> p c e", p=P))
            nc.gpsimd.dma_start(wk_bf, wk.rearrange("(c p) e -> p c e", p=P))
            nc.scalar.dma_start(wv_bf, wv.rearrange("(c p) e -> p c e", p=P))

            xts = {}

            def emit_transposes(sl):
                s0 = sl * S_SLICE
                x1t = xtpool.tile([P, DC, S_SLICE], bf16, name="x1t", tag="x1t")
                x2t = xtpool.tile([P, DC, S_SLICE], bf16, name="x2t", tag="x2t")
                xts[sl] = (x1t, x2t)
                for x_ap, xt, nm in ((x1, x1t, "x1s"), (x2, x2t, "x2s")):
                    xs3 = xstage.tile([P, S_SLICE // P, D], bf16, name=nm,
                                      tag="xs", bufs=4)
                    nc.sync.dma_start(
                        xs3, x_ap[s0:s0 + S_SLICE, :].rearrange(
                            "(r p) d -> p r d", p=P))
                    for j in range(S_SLICE // P):
                        xs = xs3[:, j, :]
                        for dq in range(DC // 4):
                            # 4 transposes share one psum tile -> 1 DVE evac.
                            # Regular matmul vs identity (out = x_blk.T @ I)
                            # streams through the normal MM pipeline — faster
                            # than transpose-mode and it keeps HAM warm.
                            pt = tpsum.tile([P, 4, P], f32, name="pt", tag="pt")
                            for di in range(4):
                                dc = dq * 4 + di
                                nc.tensor.matmul(
                                    pt[:, di, :],
                                    lhsT=xs[:, dc * P:(dc + 1) * P],
                                    rhs=ident_bf, start=True, stop=True)
                            nc.vector.tensor_copy(
                                out=xt[:, dq * 4:(dq + 1) * 4, j * P:(j + 1) * P],
                                in_=pt)

            def emit_projections(sl):
                s0 = sl * S_SLICE
                x1t, x2t = xts.pop(sl)
                for w_bf, dstT, src, scale in ((wq_bf, qT, x1t, QSCALE),
                                               (wk_bf, kT, x2t, None)):
                    for ec in range(EC):
                        pq = ppsum.tile([P, S_SLICE], f32, name="pq", tag="pp")
                        for dc in range(DC):
                            nc.tensor.matmul(
                                pq, lhsT=w_bf[:, dc, ec * P:(ec + 1) * P],
                                rhs=src[:, dc, :],
                                start=(dc == 0), stop=(dc == DC - 1))
                        if scale is not None:
                            nc.scalar.activation(
                                out=dstT[:, ec, s0:s0 + S_SLICE], in_=pq,
                                func=AF.Copy, scale=scale)
                        else:
                            nc.scalar.activation(
                                out=dstT[:, ec, s0:s0 + S_SLICE], in_=pq,
                                func=AF.Copy)
                for j in range(S_SLICE // P):
                    for eh in range(2):
                        pv = ppsum.tile([P, 512], f32, name="pv", tag="pp")
                        for dc in range(DC):
                            nc.tensor.matmul(
                                pv, lhsT=x2t[:, dc, j * P:(j + 1) * P],
                                rhs=wv_bf[:, dc, eh * 512:(eh + 1) * 512],
                                start=(dc == 0), stop=(dc == DC - 1))
                        nc.scalar.activation(
                            out=V[:, sl * (S_SLICE // P) + j, eh * 512:(eh + 1) * 512],
                            in_=pv, func=AF.Copy)

            # transposes (no weight dependency) run LAG slices ahead so the
            # PE has work while the weights stream in.
            LAG = 2
            for sl in range(NSL + LAG):
                if sl < NSL:
                    emit_transposes(sl)
                if sl == 0:
                    emit_mask_prep(tpsum)
                if sl >= LAG:
                    emit_projections(sl - LAG)

        # ================= phase 2: attention =================
        wopool = ctx.enter_context(tc.tile_pool(name="wopool", bufs=1))
        wo_bf = wopool.tile([P, DC, D], bf16, name="wo_bf")
        epool = ctx.enter_context(tc.tile_pool(name="epool", bufs=3))
        zpool = ctx.enter_context(tc.tile_pool(name="zpool", bufs=2))
        opool = ctx.enter_context(tc.tile_pool(name="opool", bufs=3))
        rpool = ctx.enter_context(tc.tile_pool(name="rpool", bufs=2))
        spsum = ctx.enter_context(tc.tile_pool(name="spsum", bufs=2, space="PSUM"))
        ypsum = ctx.enter_context(tc.tile_pool(name="ypsum", bufs=3, space="PSUM"))
        dpsum = ctx.enter_context(tc.tile_pool(name="dpsum", bufs=1, space="PSUM"))
        opsum = ctx.enter_context(tc.tile_pool(name="opsum", bufs=2, space="PSUM"))

        for dc in range(DC):
            nc.gpsimd.dma_start(wo_bf[:, dc, :], wo[dc * P:(dc + 1) * P, :])

        for qt in range(NQT):
            q0 = qt * Q_TILE
            expt = epool.tile([P, SC, Q_TILE], bf16, name="expt", tag="expt")
            for kc in range(SC):
                ps = spsum.tile([P, Q_TILE], f32, name="ps", tag="ps")
                for ec in range(EC):
                    nc.tensor.matmul(
                        ps, lhsT=kT[:, ec, kc * P:(kc + 1) * P],
                        rhs=qT[:, ec, q0:q0 + Q_TILE],
                        start=(ec == 0), stop=(ec == EC - 1))
                nc.scalar.activation(
                    out=expt[:, kc, :], in_=ps, func=AF.Exp,
                    bias=maskbias[:, kc:kc + 1], scale=1.0)

            # NOTE: matmul start=True clears has_written bits for the WHOLE
            # PSUM bank, so accumulation chains must not interleave within a
            # bank: run one chain per psum tile to completion before starting
            # the next chain that shares its bank.
            zt = zpool.tile([P, EC, Q_TILE], bf16, name="zt", tag="zt")
            for ec in range(EC):
                py = ypsum.tile([P, Q_TILE], f32, name="py", tag="py")
                for kc in range(SC):
                    nc.tensor.matmul(
                        py, lhsT=V[:, kc, ec * P:(ec + 1) * P],
                        rhs=expt[:, kc, :],
                        start=(kc == 0), stop=(kc == SC - 1))
                nc.scalar.activation(out=zt[:, ec, :], in_=py, func=AF.Relu)

            pd = dpsum.tile([P, Q_TILE // P], f32, name="pd", tag="pd")
            for qs in range(Q_TILE // P):
                for kc in range(SC):
                    nc.tensor.matmul(
                        pd[:, qs:qs + 1],
                        lhsT=expt[:, kc, qs * P:(qs + 1) * P], rhs=ones_bf,
                        start=(kc == 0), stop=(kc == SC - 1))
            recip = rpool.tile([P, Q_TILE // P], f32, name="recip", tag="recip")
            nc.vector.reciprocal(recip, pd)

            for qs in range(Q_TILE // P):
                osb = opool.tile([P, D], f32, name="osb", tag="osb")
                for fh in range(2):
                    po = opsum.tile([P, 512], f32, name="po", tag="po")
                    for ec in range(EC):
                        nc.tensor.matmul(
                            po, lhsT=zt[:, ec, qs * P:(qs + 1) * P],
                            rhs=wo_bf[:, ec, fh * 512:(fh + 1) * 512],
                            start=(ec == 0), stop=(ec == EC - 1))
                    nc.scalar.activation(
                        out=osb[:, fh * 512:(fh + 1) * 512], in_=po,
                        func=AF.Copy, scale=recip[:, qs:qs + 1])
                    # per-half store: the DMA overlaps the other half's evac
                    nc.sync.dma_start(
                        out[q0 + qs * P: q0 + (qs + 1) * P,
                            fh * 512:(fh + 1) * 512],
                        osb[:, fh * 512:(fh + 1) * 512])

    nc.compile()
    return nc


def kernel(x1, x2, maskSeq, wq, wk, wv, wo, **_unused):
    from concourse.bass_utils import run_bass_kernel_spmd

    global _cached_nc
    if _cached_nc is None:
        _cached_nc = _build()
    nc = _cached_nc

    import ml_dtypes
    bf = ml_dtypes.bfloat16
    x1 = np.ascontiguousarray(np.asarray(x1, dtype=np.float32).astype(bf))
    x2 = np.ascontiguousarray(np.asarray(x2, dtype=np.float32).astype(bf))
    maskSeq = np.ascontiguousarray(np.asarray(maskSeq, dtype=np.int32))
    wq = np.ascontiguousarray(np.asarray(wq, dtype=np.float32).astype(bf))
    wk = np.ascontiguousarray(np.asarray(wk, dtype=np.float32).astype(bf))
    wv = np.ascontiguousarray(np.asarray(wv, dtype=np.float32).astype(bf))
    wo = np.ascontiguousarray(np.asarray(wo, dtype=np.float32).astype(bf))

    in_maps = [
        {"x1": x1[c], "x2": x2[c], "msk": maskSeq[c],
         "wq": wq, "wk": wk, "wv": wv, "wo": wo}
        for c in range(N_CORES)
    ]
    res = run_bass_kernel_spmd(nc, in_maps, core_ids=list(range(N_CORES)))
    return np.stack([res.results[c]["out"] for c in range(N_CORES)], axis=0)


# revision 54
# speedup vs baseline: 1.0099x; 1.0070x over previous
"""Bass/Tile Trainium2 kernel for nn_Attention_7284264534326.

Single-head attention, B=8, S=2048, D=1024:
    q = (x1 @ wq) * D**-0.5 ; k = x2 @ wk ; v = x2 @ wv
    a = softmax(q @ k^T + mask * -1e9, axis=-1)
    out = relu(a @ v) @ wo

Sharding: data-parallel over batch; one batch element per NeuronCore (8 cores).
Each core runs the identical program on its own x1[b], x2[b], maskSeq[b].

Per-core dataflow (all matmul operands bf16, PSUM accumulation f32):
  - x1, x2 are PE-transposed into X1T/X2T ([d, s] layout, bf16).
  - qT[e,s] = wq^T-free matmul(lhsT=wq[d,e], rhs=X1T[d,s]) with 1/32 scale on
    PSUM evacuation; kT likewise; V[s,e] = matmul(lhsT=X2T[d,s], rhs=wv[d,e]).
  - scores^T[k,q] = matmul(lhsT=kT, rhs=qT); exp fused into ACT evacuation
    with the additive mask as a per-partition bias (exp(-1e9) == 0).
  - yU^T[e,q] = matmul(lhsT=V, rhs=exp^T); denom via matmul with ones-vector.
  - z^T = relu(yU^T) (normalization deferred: relu(y/d) == relu(y)/d for d>0).
  - out[q,f] = matmul(lhsT=z^T, rhs=wo) scaled by 1/denom on evacuation.
"""

import numpy as np
from contextlib import ExitStack

B, S, D = 8, 2048, 1024
P = 128
DC = D // P       # 8 chunks of the depth/contraction dim
EC = D // P       # 8 chunks of the embedding dim
SC = S // P       # 16 chunks of the sequence dim
Q_TILE = 256      # queries per attention tile
NQT = S // Q_TILE # 8
S_SLICE = 256     # seq rows per projection slice
NSL = S // S_SLICE
N_CORES = 8
QSCALE = float(D) ** -0.5  # 1/32

_cached_nc = None


def _build():
    import concourse.tile as tile
    from concourse import bacc, mybir
    from concourse.masks import make_identity

    f32 = mybir.dt.float32
    bf16 = mybir.dt.bfloat16
    i32 = mybir.dt.int32
    AF = mybir.ActivationFunctionType

    nc = bacc.Bacc("TRN2", target_bir_lowering=False, debug=False,
                   enable_asserts=False, num_devices=N_CORES)

    x1 = nc.dram_tensor("x1", [S, D], bf16, kind="ExternalInput").ap()
    x2 = nc.dram_tensor("x2", [S, D], bf16, kind="ExternalInput").ap()
    msk = nc.dram_tensor("msk", [1, S], i32, kind="ExternalInput").ap()
    wq = nc.dram_tensor("wq", [D, D], bf16, kind="ExternalInput").ap()
    wk = nc.dram_tensor("wk", [D, D], bf16, kind="ExternalInput").ap()
    wv = nc.dram_tensor("wv", [D, D], bf16, kind="ExternalInput").ap()
    wo = nc.dram_tensor("wo", [D, D], bf16, kind="ExternalInput").ap()
    out = nc.dram_tensor("out", [S, D], f32, kind="ExternalOutput").ap()

    with tile.TileContext(nc) as tc, ExitStack() as ctx:
        persist = ctx.enter_context(tc.tile_pool(name="persist", bufs=1))

        qT = persist.tile([P, EC, S], bf16, name="qT")        # [e, s] by e-chunk
        kT = persist.tile([P, EC, S], bf16, name="kT")
        V = persist.tile([P, SC, D], bf16, name="V")          # [s, e] by s-chunk
        ident = persist.tile([P, P], f32, name="ident")
        ident_bf = persist.tile([P, P], bf16, name="ident_bf")
        ones_bf = persist.tile([P, 1], bf16, name="ones_bf")
        maskbias = persist.tile([P, SC], f32, name="maskbias")

        make_identity(nc, ident_bf)
        nc.vector.memset(ones_bf, 1.0)

        def emit_mask_prep(psum_pool):
            # mask -> per-partition additive bias, [s] laid out as [P, SC]
            make_identity(nc, ident)
            with tc.tile_pool(name="mprep", bufs=1) as mprep:
                mint = mprep.tile([SC, P], i32, name="mint")
                nc.sync.dma_start(mint, msk.rearrange("o (c p) -> (o c) p", p=P))
                mf = mprep.tile([SC, P], f32, name="mf")
                nc.vector.tensor_copy(out=mf, in_=mint)       # int32 -> f32
                mneg = mprep.tile([SC, P], f32, name="mneg")
                nc.scalar.mul(mneg, mf, -1.0e9)
                mp = psum_pool.tile([P, SC], f32, name="mp", tag="pt")
                nc.tensor.transpose(mp, mneg, ident[:SC, :SC])  # [SC,P]->[P,SC]
                nc.vector.tensor_copy(out=maskbias, in_=mp)

        # ================= phase 1: projections =================
        with ExitStack() as pctx:
            wpool = pctx.enter_context(tc.tile_pool(name="wpool", bufs=1))
            xstage = pctx.enter_context(tc.tile_pool(name="xstage", bufs=2))
            xtpool = pctx.enter_context(tc.tile_pool(name="xtpool", bufs=5))
            tpsum = pctx.enter_context(tc.tile_pool(name="tpsum", bufs=4, space="PSUM"))
            ppsum = pctx.enter_context(tc.tile_pool(name="ppsum", bufs=4, space="PSUM"))

            # weights arrive as bf16 from the host: direct DMA into place,
            # wq on the scalar ring, wk/wv on gpsimd — parallel to x on sync.
            wq_bf = wpool.tile([P, DC, D], bf16, name="wq_bf")
            wk_bf = wpool.tile([P, DC, D], bf16, name="wk_bf")
            wv_bf = wpool.tile([P, DC, D], bf16, name="wv_bf")
            # each weight split across two rings so every projection's weight
            # lands early; sync stays x-first (slice-0 x DMAs emitted below
            # race ahead only on the scalar/gpsimd/vector rings).
            nc.gpsimd.dma_start(wq_bf, wq.rearrange("(c p) e -> p c e", p=P))
            nc.gpsimd.dma_start(wk_bf, wk.rearrange("(c p) e -> p c e", p=P))
            nc.gpsimd.dma_start(wv_bf, wv.rearrange("(c p) e -> p c e", p=P))

            xts = {}

            def emit_transposes(sl):
                s0 = sl * S_SLICE
                x1t = xtpool.tile([P, DC, S_SLICE], bf16, name="x1t", tag="x1t")
                x2t = xtpool.tile([P, DC, S_SLICE], bf16, name="x2t", tag="x2t")
                xts[sl] = (x1t, x2t)
                for x_ap, xt, nm, eng in ((x1, x1t, "x1s", nc.sync),
                                          (x2, x2t, "x2s", nc.scalar)):
                    xs3 = xstage.tile([P, S_SLICE // P, D], bf16, name=nm,
                                      tag="xs", bufs=4)
                    if sl == 0:
                        # split the very first loads so transposes start early
                        for r in range(S_SLICE // P):
                            eng.dma_start(
                                xs3[:, r, :],
                                x_ap[s0 + r * P: s0 + (r + 1) * P, :])
                    else:
                        eng.dma_start(
                            xs3, x_ap[s0:s0 + S_SLICE, :].rearrange(
                                "(r p) d -> p r d", p=P))
                    for j in range(S_SLICE // P):
                        xs = xs3[:, j, :]
                        for dq in range(DC // 4):
                            # 4 transposes share one psum tile -> 1 DVE evac.
                            # Regular matmul vs identity (out = x_blk.T @ I)
                            # streams through the normal MM pipeline — faster
                            # than transpose-mode and it keeps HAM warm.
                            pt = tpsum.tile([P, 4, P], f32, name="pt", tag="pt")
                            for di in range(4):
                                dc = dq * 4 + di
                                nc.tensor.matmul(
                                    pt[:, di, :],
                                    lhsT=xs[:, dc * P:(dc + 1) * P],
                                    rhs=ident_bf, start=True, stop=True)
                            nc.vector.tensor_copy(
                                out=xt[:, dq * 4:(dq + 1) * 4, j * P:(j + 1) * P],
                                in_=pt)

            def emit_projections(sl):
                s0 = sl * S_SLICE
                x1t, x2t = xts.pop(sl)
                for w_bf, dstT, src, scale in ((wq_bf, qT, x1t, QSCALE),
                                               (wk_bf, kT, x2t, None)):
                    for ec in range(EC):
                        pq = ppsum.tile([P, S_SLICE], f32, name="pq", tag="pp")
                        for dc in range(DC):
                            nc.tensor.matmul(
                                pq, lhsT=w_bf[:, dc, ec * P:(ec + 1) * P],
                                rhs=src[:, dc, :],
                                start=(dc == 0), stop=(dc == DC - 1))
                        if scale is not None:
                            nc.scalar.activation(
                                out=dstT[:, ec, s0:s0 + S_SLICE], in_=pq,
                                func=AF.Copy, scale=scale)
                        else:
                            nc.scalar.activation(
                                out=dstT[:, ec, s0:s0 + S_SLICE], in_=pq,
                                func=AF.Copy)
                for j in range(S_SLICE // P):
                    for eh in range(2):
                        pv = ppsum.tile([P, 512], f32, name="pv", tag="pp")
                        for dc in range(DC):
                            nc.tensor.matmul(
                                pv, lhsT=x2t[:, dc, j * P:(j + 1) * P],
                                rhs=wv_bf[:, dc, eh * 512:(eh + 1) * 512],
                                start=(dc == 0), stop=(dc == DC - 1))
                        nc.scalar.activation(
                            out=V[:, sl * (S_SLICE // P) + j, eh * 512:(eh + 1) * 512],
                            in_=pv, func=AF.Copy)

            # transposes (no weight dependency) run LAG slices ahead so the
            # PE has work while the weights stream in.
            LAG = 2
            for sl in range(NSL + LAG):
                if sl < NSL:
                    emit_transposes(sl)
                if sl == 0:
                    emit_mask_prep(tpsum)
                if sl >= LAG:
                    emit_projections(sl - LAG)

        # ================= phase 2: attention =================
        wopool = ctx.enter_context(tc.tile_pool(name="wopool", bufs=1))
        wo_bf = wopool.tile([P, DC, D], bf16, name="wo_bf")
        epool = ctx.enter_context(tc.tile_pool(name="epool", bufs=3))
        zpool = ctx.enter_context(tc.tile_pool(name="zpool", bufs=2))
        opool = ctx.enter_context(tc.tile_pool(name="opool", bufs=3))
        rpool = ctx.enter_context(tc.tile_pool(name="rpool", bufs=2))
        spsum = ctx.enter_context(tc.tile_pool(name="spsum", bufs=2, space="PSUM"))
        ypsum = ctx.enter_context(tc.tile_pool(name="ypsum", bufs=3, space="PSUM"))
        dpsum = ctx.enter_context(tc.tile_pool(name="dpsum", bufs=1, space="PSUM"))
        opsum = ctx.enter_context(tc.tile_pool(name="opsum", bufs=2, space="PSUM"))

        for dc in range(DC):
            nc.gpsimd.dma_start(wo_bf[:, dc, :], wo[dc * P:(dc + 1) * P, :])

        for qt in range(NQT):
            q0 = qt * Q_TILE
            expt = epool.tile([P, SC, Q_TILE], bf16, name="expt", tag="expt")
            for kc in range(SC):
                ps = spsum.tile([P, Q_TILE], f32, name="ps", tag="ps")
                for ec in range(EC):
                    nc.tensor.matmul(
                        ps, lhsT=kT[:, ec, kc * P:(kc + 1) * P],
                        rhs=qT[:, ec, q0:q0 + Q_TILE],
                        start=(ec == 0), stop=(ec == EC - 1))
                nc.scalar.activation(
                    out=expt[:, kc, :], in_=ps, func=AF.Exp,
                    bias=maskbias[:, kc:kc + 1], scale=1.0)

            # NOTE: matmul start=True clears has_written bits for the WHOLE
            # PSUM bank, so accumulation chains must not interleave within a
            # bank: run one chain per psum tile to completion before starting
            # the next chain that shares its bank.
            zt = zpool.tile([P, EC, Q_TILE], bf16, name="zt", tag="zt")
            for ec in range(EC):
                py = ypsum.tile([P, Q_TILE], f32, name="py", tag="py")
                for kc in range(SC):
                    nc.tensor.matmul(
                        py, lhsT=V[:, kc, ec * P:(ec + 1) * P],
                        rhs=expt[:, kc, :],
                        start=(kc == 0), stop=(kc == SC - 1))
                nc.scalar.activation(out=zt[:, ec, :], in_=py, func=AF.Relu)

            pd = dpsum.tile([P, Q_TILE // P], f32, name="pd", tag="pd")
            for qs in range(Q_TILE // P):
                for kc in range(SC):
                    nc.tensor.matmul(
                        pd[:, qs:qs + 1],
                        lhsT=expt[:, kc, qs * P:(qs + 1) * P], rhs=ones_bf,
                        start=(kc == 0), stop=(kc == SC - 1))
            recip = rpool.tile([P, Q_TILE // P], f32, name="recip", tag="recip")
            nc.vector.reciprocal(recip, pd)

            for qs in range(Q_TILE // P):
                osb = opool.tile([P, D], f32, name="osb", tag="osb")
                for fh in range(2):
                    po = opsum.tile([P, 512], f32, name="po", tag="po")
                    for ec in range(EC):
                        nc.tensor.matmul(
                            po, lhsT=zt[:, ec, qs * P:(qs + 1) * P],
                            rhs=wo_bf[:, ec, fh * 512:(fh + 1) * 512],
                            start=(ec == 0), stop=(ec == EC - 1))
                    nc.scalar.activation(
                        out=osb[:, fh * 512:(fh + 1) * 512], in_=po,
                        func=AF.Copy, scale=recip[:, qs:qs + 1])
                    # per-half store: the DMA overlaps the other half's evac
                    nc.sync.dma_start(
                        out[q0 + qs * P: q0 + (qs + 1) * P,
                            fh * 512:(fh + 1) * 512],
                        osb[:, fh * 512:(fh + 1) * 512])

    nc.compile()
    return nc


def kernel(x1, x2, maskSeq, wq, wk, wv, wo, **_unused):
    from concourse.bass_utils import run_bass_kernel_spmd

    global _cached_nc
    if _cached_nc is None:
        _cached_nc = _build()
    nc = _cached_nc

    import ml_dtypes
    bf = ml_dtypes.bfloat16
    x1 = np.ascontiguousarray(np.asarray(x1, dtype=np.float32).astype(bf))
    x2 = np.ascontiguousarray(np.asarray(x2, dtype=np.float32).astype(bf))
    maskSeq = np.ascontiguousarray(np.asarray(maskSeq, dtype=np.int32))
    wq = np.ascontiguousarray(np.asarray(wq, dtype=np.float32).astype(bf))
    wk = np.ascontiguousarray(np.asarray(wk, dtype=np.float32).astype(bf))
    wv = np.ascontiguousarray(np.asarray(wv, dtype=np.float32).astype(bf))
    wo = np.ascontiguousarray(np.asarray(wo, dtype=np.float32).astype(bf))

    in_maps = [
        {"x1": x1[c], "x2": x2[c], "msk": maskSeq[c],
         "wq": wq, "wk": wk, "wv": wv, "wo": wo}
        for c in range(N_CORES)
    ]
    res = run_bass_kernel_spmd(nc, in_maps, core_ids=list(range(N_CORES)))
    return np.stack([res.results[c]["out"] for c in range(N_CORES)], axis=0)
